# revision 6
# baseline (speedup 1.0000x reference)
"""Trainium2 Bass kernel for DeformableSpatialEncoder.

Math (per frame, Lq=196 tokens on a 14x14 grid, D=768, 4 heads x 192):
  feat   = patch_embed(x)                     -> matmul after host im2col
  value  = feat @ Wv.T                        (token-major on device)
  off,aw = feat @ [Wo;Wa].T (+b), softmax(aw over 4 points)
  attn[q,(m,dh)] = sum_p aw * bilinear_sample(value_m, loc(q,m,p))
  out    = mean_q(attn) @ Wout.T + bo         (mean commutes with linear)
  final  = out @ Wproj.T + bp

Bilinear sampling is exact via "tent" weights: the weight of grid row y for
sample row-coord py is relu(1-|py-y|) (zero outside the grid == zero padding).
So per head  attnT = V.T @ Ws  with  Ws[(y,x), q] = sum_p aw * ty(y) * tx(x).
Ws is built token-major on DVE (per-partition scalars are per-token), then
transposed 128x128-blockwise on the PE.

Sharding: data-parallel over the 64 frames, 8 per core, no collectives.
"""

import os
import sys

for _p in (
    "/root/.axon_site",
    "/root/.axon_site/_ro/trn_rl_repo",
    "/root/.axon_site/_ro/pypackages",
    "/opt/trn_rl_repo",
):
    if os.path.isdir(_p) and _p not in sys.path:
        sys.path.append(_p)

import numpy as np

import concourse.bass as bass
import concourse.mybir as mybir
import concourse.tile as tile
from concourse import bacc
from concourse.masks import make_identity

F32 = mybir.dt.float32
AF = mybir.ActivationFunctionType
OP = mybir.AluOpType

HF = WF = 14
LQ = 196            # tokens per frame
D = 768
NH = 4              # heads
NP = 4              # points
DH = 192            # head dim
NFRAME = 8          # frames per core
NCORES = 8
KC = 6              # 768 / 128 contraction chunks
QCH = (128, 68)     # token-dim chunks (196 = 128 + 68)

# consts layout (one [128, NCONST] f32 tensor)
C_IOTA = 0          # cols 0..13: 0..13 in every partition
C_CX0, C_CX1 = 14, 15   # 14*gx - 0.5 per token, chunk0 rows 0..127 / chunk1 rows 0..67
C_CY0, C_CY1 = 16, 17
C_BEMB = 18         # 6 cols: embed_b  per dout-chunk
C_BOUT = 24         # 6 cols: outp_b
C_BPROJ = 30        # 6 cols: proj_b
C_VB = 36           # row 0, cols 36..803: value_b
C_OAB = 804         # row 0, cols 804..851: [off_b; aw_b]
NCONST = 852


def _build_program():
    nc = bacc.Bacc("TRN2", target_bir_lowering=False, debug=False,
                   enable_asserts=False, num_devices=1)

    xcol = nc.dram_tensor("xcol", (NFRAME, D, LQ), F32, kind="ExternalInput").ap()
    wembT = nc.dram_tensor("wembT", (D, D), F32, kind="ExternalInput").ap()
    wvalT = nc.dram_tensor("wvalT", (D, D), F32, kind="ExternalInput").ap()
    woaT = nc.dram_tensor("woaT", (D, 48), F32, kind="ExternalInput").ap()
    woutT = nc.dram_tensor("woutT", (D, D), F32, kind="ExternalInput").ap()
    wprojT = nc.dram_tensor("wprojT", (D, D), F32, kind="ExternalInput").ap()
    consts = nc.dram_tensor("consts", (128, NCONST), F32, kind="ExternalInput").ap()
    out8 = nc.dram_tensor("out8", (KC, 128, NFRAME), F32, kind="ExternalOutput").ap()

    with tile.TileContext(nc) as tc:
        _emit(tc, xcol, wembT, wvalT, woaT, woutT, wprojT, consts, out8)
    nc.compile()
    return nc


def _emit(tc, xcol, wembT, wvalT, woaT, woutT, wprojT, consts, out8):
    nc = tc.nc
    from contextlib import ExitStack
    ctx = ExitStack()
    with ctx:
        cpool = ctx.enter_context(tc.tile_pool(name="consts", bufs=1))
        wpool = ctx.enter_context(tc.tile_pool(name="weights", bufs=1))
        xpool = ctx.enter_context(tc.tile_pool(name="xin", bufs=12))
        fpool = ctx.enter_context(tc.tile_pool(name="featT", bufs=12))
        vpool = ctx.enter_context(tc.tile_pool(name="val", bufs=4))
        spool = ctx.enter_context(tc.tile_pool(name="small", bufs=8))
        gpool = ctx.enter_context(tc.tile_pool(name="wst", bufs=2))
        rpool = ctx.enter_context(tc.tile_pool(name="wsum", bufs=4))
        hpool = ctx.enter_context(tc.tile_pool(name="wsh", bufs=10))
        apool = ctx.enter_context(tc.tile_pool(name="accum", bufs=1))
        pp_a = ctx.enter_context(tc.tile_pool(name="psA", bufs=3, space="PSUM"))
        pp_t = ctx.enter_context(tc.tile_pool(name="psT", bufs=3, space="PSUM"))

        # ---- constants & weights (loaded once) ----
        cons = cpool.tile([128, NCONST], F32)
        nc.sync.dma_start(cons[:], consts)
        iden = cpool.tile([128, 128], F32)
        make_identity(nc, iden[:])
        vb_bc = cpool.tile([128, D], F32)
        nc.gpsimd.partition_broadcast(vb_bc[:], cons[0:1, C_VB:C_VB + D])
        oab_bc = cpool.tile([128, 48], F32)
        nc.gpsimd.partition_broadcast(oab_bc[:], cons[0:1, C_OAB:C_OAB + 48])

        def load_w(nm, ap_dram, width):
            ts_ = []
            for k in range(KC):
                t = wpool.tile([128, width], F32, name=f"{nm}{k}", tag=f"{nm}{k}")
                nc.sync.dma_start(t[:], ap_dram[k * 128:(k + 1) * 128, :])
                ts_.append(t)
            return ts_

        wemb = load_w("we", wembT, D)
        wval = load_w("wv", wvalT, D)
        woa = load_w("wo", woaT, 48)
        wout = load_w("wu", woutT, D)
        wproj = load_w("wp", wprojT, D)

        # attn token-sum accumulator, [768 rows over 6 tiles, 8 frames]
        asum = [apool.tile([128, NFRAME], F32, name=f"asum{t}", tag=f"as{t}")
                for t in range(KC)]

        for f in range(NFRAME):
            # ---- load im2col'd frame ----
            xin = []
            for k in range(KC):
                t = xpool.tile([128, LQ], F32)
                nc.sync.dma_start(t[:], xcol[f, k * 128:(k + 1) * 128, :])
                xin.append(t)

            # ---- patch embed: featT[dout, q] ----
            featT = []
            for m in range(KC):
                ps = pp_a.tile([128, LQ], F32, tag="mm")
                for k in range(KC):
                    nc.tensor.matmul(ps[:], wemb[k][:, m * 128:(m + 1) * 128],
                                     xin[k][:], start=(k == 0), stop=(k == KC - 1))
                ft = fpool.tile([128, LQ], F32)
                # psum->sbuf + embed bias (per-partition scalar)
                nc.vector.tensor_scalar(ft[:], ps[:], cons[:, C_BEMB + m:C_BEMB + m + 1],
                                        None, OP.add)
                featT.append(ft)

            # ---- value, token-major V[q, (m,dh)] ----
            V = []
            for qc, qn in enumerate(QCH):
                vt = vpool.tile([128, D], F32, tag="v")
                qs = slice(qc * 128, qc * 128 + qn)
                for nb in range(2):
                    ps = pp_a.tile([128, 384], F32, tag="mm")
                    for k in range(KC):
                        nc.tensor.matmul(ps[:qn, :], featT[k][:, qs],
                                         wval[k][:, nb * 384:(nb + 1) * 384],
                                         start=(k == 0), stop=(k == KC - 1))
                    nc.vector.tensor_tensor(vt[:qn, nb * 384:(nb + 1) * 384],
                                            ps[:qn, :], vb_bc[:qn, nb * 384:(nb + 1) * 384],
                                            OP.add)
                V.append(vt)

            # ---- off/aw, token-major [q, 48] ----
            WsT = []   # per q-chunk: [q, (m, y, x)] = [q, 784] summed over points
            for qc, qn in enumerate(QCH):
                qs = slice(qc * 128, qc * 128 + qn)
                ps = pp_a.tile([128, 48], F32, tag="mm")
                for k in range(KC):
                    nc.tensor.matmul(ps[:qn, :], featT[k][:, qs], woa[k][:],
                                     start=(k == 0), stop=(k == KC - 1))
                oa = spool.tile([128, 48], F32, tag="oa")
                nc.vector.tensor_tensor(oa[:qn, :], ps[:qn, :], oab_bc[:qn, :], OP.add)

                # softmax over points: aw cols 32..48 viewed [q, m, p]
                aw3 = oa[:qn, 32:48].rearrange("q (m p) -> q m p", m=NH)
                mx = spool.tile([128, NH], F32, tag="mx")
                nc.vector.tensor_reduce(mx[:qn, :], aw3, mybir.AxisListType.X, OP.max)
                ex = spool.tile([128, 16], F32, tag="ex")
                nc.vector.tensor_tensor(
                    ex[:qn, :].rearrange("q (m p) -> q m p", m=NH), aw3,
                    mx[:qn, :].unsqueeze(2).broadcast_to((qn, NH, NP)), OP.subtract)
                nc.scalar.activation(ex[:qn, :], ex[:qn, :], AF.Exp)
                sm = spool.tile([128, NH], F32, tag="sm")
                nc.vector.tensor_reduce(sm[:qn, :],
                                        ex[:qn, :].rearrange("q (m p) -> q m p", m=NH),
                                        mybir.AxisListType.X, OP.add)
                nc.vector.reciprocal(sm[:qn, :], sm[:qn, :])
                awn = spool.tile([128, 16], F32, tag="awn")
                nc.vector.tensor_tensor(
                    awn[:qn, :].rearrange("q (m p) -> q m p", m=NH),
                    ex[:qn, :].rearrange("q (m p) -> q m p", m=NH),
                    sm[:qn, :].unsqueeze(2).broadcast_to((qn, NH, NP)), OP.mult)

                # sample coords: px = offx + (14*gx - 0.5), py likewise
                cx = cons[:qn, C_CX0 + qc:C_CX0 + qc + 1]
                cy = cons[:qn, C_CY0 + qc:C_CY0 + qc + 1]
                px = spool.tile([128, 16], F32, tag="px")
                py = spool.tile([128, 16], F32, tag="py")
                offv = oa[:qn, 0:32].rearrange("q (m p c) -> q m p c", m=NH, p=NP)
                nc.vector.tensor_scalar(px[:qn, :].rearrange("q (m p) -> q m p", m=NH),
                                        offv[:, :, :, 0], cx, None, OP.add)
                nc.vector.tensor_scalar(py[:qn, :].rearrange("q (m p) -> q m p", m=NH),
                                        offv[:, :, :, 1], cy, None, OP.add)

                # tents, layout [q, (m, y|x, p)]; ty carries -tent*aw, tx carries -tent
                iota_y = cons[:qn, C_IOTA:C_IOTA + 14] \
                    .unsqueeze(1).unsqueeze(3).broadcast_to((qn, NH, 14, NP))

                def neg_tent(coord, tag):
                    d = spool.tile([128, NH * 14 * NP], F32, tag=tag)
                    d4 = d[:qn, :].rearrange("q (m y p) -> q m y p", m=NH, y=14)
                    nc.vector.tensor_tensor(
                        d4, coord[:qn, :].rearrange("q (m p) -> q m p", m=NH)
                        .unsqueeze(2).broadcast_to((qn, NH, 14, NP)),
                        iota_y, OP.subtract)
                    # |d| on ACT, then (x-1) and min(.,0) fused  ==  -relu(1-|d|)
                    nc.scalar.activation(d[:qn, :], d[:qn, :], AF.Abs)
                    nc.vector.tensor_scalar(d[:qn, :], d[:qn, :], 1.0, 0.0,
                                            OP.subtract, OP.min)
                    return d

                ty = neg_tent(py, "ty")
                tx = neg_tent(px, "tx")
                # fold attention weights into ty
                nc.vector.tensor_tensor(
                    ty[:qn, :].rearrange("q (m y p) -> q m y p", m=NH, y=14),
                    ty[:qn, :].rearrange("q (m y p) -> q m y p", m=NH, y=14),
                    awn[:qn, :].rearrange("q (m p) -> q m p", m=NH)
                    .unsqueeze(2).broadcast_to((qn, NH, 14, NP)), OP.mult)

                # Ws token-major: prod over (y,x) then sum over p
                wst = gpool.tile([128, NH * 196 * NP], F32, tag="wst")
                nc.vector.tensor_tensor(
                    wst[:qn, :].rearrange("q (m y x p) -> q m y x p", m=NH, y=14, x=14),
                    ty[:qn, :].rearrange("q (m y p) -> q m y p", m=NH, y=14)
                    .unsqueeze(3).broadcast_to((qn, NH, 14, 14, NP)),
                    tx[:qn, :].rearrange("q (m x p) -> q m x p", m=NH, x=14)
                    .unsqueeze(2).broadcast_to((qn, NH, 14, 14, NP)), OP.mult)
                wsum = rpool.tile([128, NH * 196], F32, tag="wsum")
                nc.vector.tensor_reduce(wsum[:qn, :],
                                        wst[:qn, :].rearrange("q (g p) -> q g p", p=NP),
                                        mybir.AxisListType.X, OP.add)
                WsT.append(wsum)

            # ---- per head: transpose Ws, gather matmuls, token-sum ----
            for h in range(NH):
                # Ws[hw, q] tiles per hw-chunk
                wsh = [hpool.tile([128, LQ], F32, name=f"wsh{f}_{h}_{i}", tag="wsh")
                       for i in range(2)]
                for hc, hn in enumerate(QCH):
                    for qc, qn in enumerate(QCH):
                        tp = pp_t.tile([128, 128], F32, tag="tr")
                        nc.tensor.transpose(
                            tp[:hn, :qn],
                            WsT[qc][:qn, h * 196 + hc * 128: h * 196 + hc * 128 + hn],
                            iden[:qn, :qn])
                        nc.vector.tensor_copy(wsh[hc][:hn, qc * 128:qc * 128 + qn],
                                              tp[:hn, :qn])
                # attnT pieces: dh split 128+64 to keep PSUM partitions aligned
                for mc, (m0, mn) in enumerate(((0, 128), (128, 64))):
                    ps = pp_a.tile([128, LQ], F32, tag="mm")
                    for hc, hn in enumerate(QCH):
                        nc.tensor.matmul(
                            ps[:mn, :],
                            V[hc][:hn, h * DH + m0: h * DH + m0 + mn],
                            wsh[hc][:hn, :], start=(hc == 0), stop=(hc == 1))
                    # token-sum into asum rows g0..g0+mn  (g = h*192 + m0 + j)
                    g0 = h * DH + m0
                    r = 0
                    while r < mn:
                        t_i, row = divmod(g0 + r, 128)
                        take = min(mn - r, 128 - row)
                        nc.vector.tensor_reduce(
                            asum[t_i][row:row + take, f:f + 1],
                            ps[r:r + take, :], mybir.AxisListType.X, OP.add)
                        r += take

        # ---- tail: pooled = asum @ woutT (+bo), final = pooled @ wprojT (+bp) ----
        pooled = []
        for m in range(KC):
            ps = pp_a.tile([128, NFRAME], F32, tag="mm")
            for k in range(KC):
                nc.tensor.matmul(ps[:], wout[k][:, m * 128:(m + 1) * 128],
                                 asum[k][:], start=(k == 0), stop=(k == KC - 1))
            pt = spool.tile([128, NFRAME], F32, tag="pool")
            nc.vector.tensor_scalar(pt[:], ps[:], cons[:, C_BOUT + m:C_BOUT + m + 1],
                                    None, OP.add)
            pooled.append(pt)
        for m in range(KC):
            ps = pp_a.tile([128, NFRAME], F32, tag="mm")
            for k in range(KC):
                nc.tensor.matmul(ps[:], wproj[k][:, m * 128:(m + 1) * 128],
                                 pooled[k][:], start=(k == 0), stop=(k == KC - 1))
            ot = spool.tile([128, NFRAME], F32, tag="outt")
            nc.vector.tensor_scalar(ot[:], ps[:], cons[:, C_BPROJ + m:C_BPROJ + m + 1],
                                    None, OP.add)
            nc.sync.dma_start(out8[m], ot[:])


def _host_prep(inputs):
    x = np.asarray(inputs["x"], dtype=np.float32)
    n = x.shape[0] * x.shape[1]
    xi = x.reshape(n, 3, HF, 16, WF, 16)
    xcol = np.ascontiguousarray(xi.transpose(0, 1, 3, 5, 2, 4)).reshape(n, D, LQ)

    wembT = np.ascontiguousarray(
        np.asarray(inputs["embed_w"], np.float32).reshape(D, D).T)
    wvalT = np.ascontiguousarray(np.asarray(inputs["value_w"], np.float32).T)
    woaT = np.ascontiguousarray(np.concatenate(
        [np.asarray(inputs["off_w"], np.float32),
         np.asarray(inputs["aw_w"], np.float32)], axis=0).T)
    woutT = np.ascontiguousarray(
        (np.asarray(inputs["outp_w"], np.float32) / LQ).T)
    wprojT = np.ascontiguousarray(np.asarray(inputs["proj_w"], np.float32).T)

    cons = np.zeros((128, NCONST), np.float32)
    cons[:, C_IOTA:C_IOTA + 14] = np.arange(14, dtype=np.float32)[None, :]
    lin = np.linspace(0.0, 1.0, HF, dtype=np.float32)
    cxf = (14.0 * lin[np.arange(LQ) % WF] - 0.5).astype(np.float32)
    cyf = (14.0 * lin[np.arange(LQ) // WF] - 0.5).astype(np.float32)
    cons[:, C_CX0] = cxf[:128]
    cons[:68, C_CX1] = cxf[128:]
    cons[:, C_CY0] = cyf[:128]
    cons[:68, C_CY1] = cyf[128:]
    cons[:, C_BEMB:C_BEMB + 6] = np.asarray(inputs["embed_b"], np.float32).reshape(6, 128).T
    cons[:, C_BOUT:C_BOUT + 6] = np.asarray(inputs["outp_b"], np.float32).reshape(6, 128).T
    cons[:, C_BPROJ:C_BPROJ + 6] = np.asarray(inputs["proj_b"], np.float32).reshape(6, 128).T
    cons[0, C_VB:C_VB + D] = np.asarray(inputs["value_b"], np.float32)
    cons[0, C_OAB:C_OAB + 48] = np.concatenate(
        [np.asarray(inputs["off_b"], np.float32),
         np.asarray(inputs["aw_b"], np.float32)])

    shared = dict(wembT=wembT, wvalT=wvalT, woaT=woaT, woutT=woutT,
                  wprojT=wprojT, consts=cons)
    in_maps = [dict(shared, xcol=np.ascontiguousarray(
        xcol[c * NFRAME:(c + 1) * NFRAME])) for c in range(NCORES)]
    return in_maps


_NC_CACHE = None


def _get_nc():
    global _NC_CACHE
    if _NC_CACHE is None:
        _NC_CACHE = _build_program()
    return _NC_CACHE


def kernel(**inputs) -> np.ndarray:
    from concourse.bass_utils import run_bass_kernel_spmd
    nc = _get_nc()
    in_maps = _host_prep(inputs)
    res = run_bass_kernel_spmd(nc, in_maps, list(range(NCORES))).results
    outs = []
    for c in range(NCORES):
        o = res[c]["out8"]              # (6, 128, 8)
        outs.append(o.transpose(2, 0, 1).reshape(NFRAME, D))
    full = np.concatenate(outs, axis=0)  # (64, 768)
    return full.reshape(4, 16, D).astype(np.float32)


# revision 9
# speedup vs baseline: 21.3633x; 21.3633x over previous
"""Trainium2 Bass kernel for DeformableSpatialEncoder.

Math (per frame, Lq=196 tokens on a 14x14 grid, D=768, 4 heads x 192):
  feat   = patch_embed(x)                     -> matmul after host im2col
  value  = feat @ Wv.T                        (token-major on device)
  off,aw = feat @ [Wo;Wa].T (+b), softmax(aw over 4 points)
  attn[q,(m,dh)] = sum_p aw * bilinear_sample(value_m, loc(q,m,p))
  out    = mean_q(attn) @ Wout.T + bo         (mean commutes with linear)
  final  = out @ Wproj.T + bp

Bilinear sampling is exact via "tent" weights: the weight of grid row y for
sample row-coord py is relu(1-|py-y|) (zero outside the grid == zero padding).
So per head  attnT = V.T @ Ws  with  Ws[(y,x), q] = sum_p aw * ty(y) * tx(x).
Ws is built token-major on DVE (per-partition scalars are per-token), then
transposed 128x128-blockwise on the PE.

Sharding: data-parallel over the 64 frames, 8 per core, no collectives.
"""

import os
import sys

for _p in (
    "/root/.axon_site",
    "/root/.axon_site/_ro/trn_rl_repo",
    "/root/.axon_site/_ro/pypackages",
    "/opt/trn_rl_repo",
):
    if os.path.isdir(_p) and _p not in sys.path:
        sys.path.append(_p)

import numpy as np

import concourse.bass as bass
import concourse.mybir as mybir
import concourse.tile as tile
from concourse import bacc
from concourse.masks import make_identity

F32 = mybir.dt.float32
AF = mybir.ActivationFunctionType
OP = mybir.AluOpType

HF = WF = 14
LQ = 196            # tokens per frame
D = 768
NH = 4              # heads
NP = 4              # points
DH = 192            # head dim
NFRAME = 8          # frames per core
NCORES = 8
KC = 6              # 768 / 128 contraction chunks
QCH = (128, 68)     # token-dim chunks (196 = 128 + 68)

# consts layout (one [128, NCONST] f32 tensor)
C_IOTA = 0          # cols 0..13: 0..13 in every partition
C_CX0, C_CX1 = 14, 15   # 14*gx - 0.5 per token, chunk0 rows 0..127 / chunk1 rows 0..67
C_CY0, C_CY1 = 16, 17
C_BEMB = 18         # 6 cols: embed_b  per dout-chunk
C_BOUT = 24         # 6 cols: outp_b
C_BPROJ = 30        # 6 cols: proj_b
C_VB = 36           # row 0, cols 36..803: value_b
C_OAB = 804         # row 0, cols 804..851: [off_b; aw_b]
NCONST = 852


def _build_program(reps=1):
    nc = bacc.Bacc("TRN2", target_bir_lowering=False, debug=False,
                   enable_asserts=False, num_devices=1)

    xcol = nc.dram_tensor("xcol", (NFRAME, D, LQ), F32, kind="ExternalInput").ap()
    wvalT = nc.dram_tensor("wvalT", (D, D), F32, kind="ExternalInput").ap()
    woaT = nc.dram_tensor("woaT", (D, 48), F32, kind="ExternalInput").ap()
    woutT = nc.dram_tensor("woutT", (D, D), F32, kind="ExternalInput").ap()
    wprojT = nc.dram_tensor("wprojT", (D, D), F32, kind="ExternalInput").ap()
    consts = nc.dram_tensor("consts", (128, NCONST), F32, kind="ExternalInput").ap()
    out8 = nc.dram_tensor("out8", (KC, 128, NFRAME), F32, kind="ExternalOutput").ap()

    with tile.TileContext(nc) as tc:
        _emit(tc, xcol, wvalT, woaT, woutT, wprojT, consts, out8, reps)
    nc.compile()
    return nc


def _emit(tc, xcol, wvalT, woaT, woutT, wprojT, consts, out8, reps=1):
    nc = tc.nc
    from contextlib import ExitStack
    ctx = ExitStack()
    with ctx:
        cpool = ctx.enter_context(tc.tile_pool(name="consts", bufs=1))
        wpool = ctx.enter_context(tc.tile_pool(name="weights", bufs=1))
        xpool = ctx.enter_context(tc.tile_pool(name="xin", bufs=12))
        vpool = ctx.enter_context(tc.tile_pool(name="val", bufs=4))
        spool = ctx.enter_context(tc.tile_pool(name="small", bufs=8))
        gpool = ctx.enter_context(tc.tile_pool(name="wst", bufs=2))
        rpool = ctx.enter_context(tc.tile_pool(name="wsum", bufs=4))
        hpool = ctx.enter_context(tc.tile_pool(name="wsh", bufs=10))
        apool = ctx.enter_context(tc.tile_pool(name="accum", bufs=1))
        pp_a = ctx.enter_context(tc.tile_pool(name="psA", bufs=3, space="PSUM"))
        pp_t = ctx.enter_context(tc.tile_pool(name="psT", bufs=3, space="PSUM"))

        # ---- constants & weights (loaded once) ----
        cons = cpool.tile([128, NCONST], F32)
        nc.sync.dma_start(cons[:], consts)
        iden = cpool.tile([128, 128], F32)
        make_identity(nc, iden[:])
        vb_bc = cpool.tile([128, D], F32)
        nc.gpsimd.partition_broadcast(vb_bc[:], cons[0:1, C_VB:C_VB + D])
        oab_bc = cpool.tile([128, 48], F32)
        nc.gpsimd.partition_broadcast(oab_bc[:], cons[0:1, C_OAB:C_OAB + 48])

        def load_w(nm, ap_dram, width):
            ts_ = []
            for k in range(KC):
                t = wpool.tile([128, width], F32, name=f"{nm}{k}", tag=f"{nm}{k}")
                nc.sync.dma_start(t[:], ap_dram[k * 128:(k + 1) * 128, :])
                ts_.append(t)
            return ts_

        wval = load_w("wv", wvalT, D)
        woa = load_w("wo", woaT, 48)
        wout = load_w("wu", woutT, D)
        wproj = load_w("wp", wprojT, D)

        # attn token-sum accumulator, [768 rows over 6 tiles, 8 frames]
        asum = [apool.tile([128, NFRAME], F32, name=f"asum{t}", tag=f"as{t}")
                for t in range(KC)]

        for f in [fr for _ in range(reps) for fr in range(NFRAME)]:
            # ---- load im2col'd frame ----
            xin = []
            for k in range(KC):
                t = xpool.tile([128, LQ], F32)
                nc.sync.dma_start(t[:], xcol[f, k * 128:(k + 1) * 128, :])
                xin.append(t)

            # ---- value, token-major V[q, (m,dh)]; embed weight folded on host ----
            V = []
            for qc, qn in enumerate(QCH):
                vt = vpool.tile([128, D], F32, tag="v")
                qs = slice(qc * 128, qc * 128 + qn)
                for nb in range(2):
                    ps = pp_a.tile([128, 384], F32, tag="mm")
                    for k in range(KC):
                        nc.tensor.matmul(ps[:qn, :], xin[k][:, qs],
                                         wval[k][:, nb * 384:(nb + 1) * 384],
                                         start=(k == 0), stop=(k == KC - 1))
                    nc.vector.tensor_tensor(vt[:qn, nb * 384:(nb + 1) * 384],
                                            ps[:qn, :], vb_bc[:qn, nb * 384:(nb + 1) * 384],
                                            OP.add)
                V.append(vt)

            # ---- off/aw, token-major [q, 48] ----
            WsT = []   # per q-chunk: [q, (m, y, x)] = [q, 784] summed over points
            for qc, qn in enumerate(QCH):
                qs = slice(qc * 128, qc * 128 + qn)
                ps = pp_a.tile([128, 48], F32, tag="mm")
                for k in range(KC):
                    nc.tensor.matmul(ps[:qn, :], xin[k][:, qs], woa[k][:],
                                     start=(k == 0), stop=(k == KC - 1))
                oa = spool.tile([128, 48], F32, tag="oa")
                nc.vector.tensor_tensor(oa[:qn, :], ps[:qn, :], oab_bc[:qn, :], OP.add)

                # softmax over points: aw cols 32..48 viewed [q, m, p]
                aw3 = oa[:qn, 32:48].rearrange("q (m p) -> q m p", m=NH)
                mx = spool.tile([128, NH], F32, tag="mx")
                nc.vector.tensor_reduce(mx[:qn, :], aw3, mybir.AxisListType.X, OP.max)
                ex = spool.tile([128, 16], F32, tag="ex")
                nc.vector.tensor_tensor(
                    ex[:qn, :].rearrange("q (m p) -> q m p", m=NH), aw3,
                    mx[:qn, :].unsqueeze(2).broadcast_to((qn, NH, NP)), OP.subtract)
                nc.scalar.activation(ex[:qn, :], ex[:qn, :], AF.Exp)
                sm = spool.tile([128, NH], F32, tag="sm")
                nc.vector.tensor_reduce(sm[:qn, :],
                                        ex[:qn, :].rearrange("q (m p) -> q m p", m=NH),
                                        mybir.AxisListType.X, OP.add)
                nc.vector.reciprocal(sm[:qn, :], sm[:qn, :])
                awn = spool.tile([128, 16], F32, tag="awn")
                nc.vector.tensor_tensor(
                    awn[:qn, :].rearrange("q (m p) -> q m p", m=NH),
                    ex[:qn, :].rearrange("q (m p) -> q m p", m=NH),
                    sm[:qn, :].unsqueeze(2).broadcast_to((qn, NH, NP)), OP.mult)

                # sample coords: px = offx + (14*gx - 0.5), py likewise
                cx = cons[:qn, C_CX0 + qc:C_CX0 + qc + 1]
                cy = cons[:qn, C_CY0 + qc:C_CY0 + qc + 1]
                px = spool.tile([128, 16], F32, tag="px")
                py = spool.tile([128, 16], F32, tag="py")
                offv = oa[:qn, 0:32].rearrange("q (m p c) -> q m p c", m=NH, p=NP)
                nc.vector.tensor_scalar(px[:qn, :].rearrange("q (m p) -> q m p", m=NH),
                                        offv[:, :, :, 0], cx, None, OP.add)
                nc.vector.tensor_scalar(py[:qn, :].rearrange("q (m p) -> q m p", m=NH),
                                        offv[:, :, :, 1], cy, None, OP.add)

                # tents, layout [q, (m, y|x, p)]; ty carries -tent*aw, tx carries -tent
                iota_y = cons[:qn, C_IOTA:C_IOTA + 14] \
                    .unsqueeze(1).unsqueeze(3).broadcast_to((qn, NH, 14, NP))

                def neg_tent(coord, tag):
                    d = spool.tile([128, NH * 14 * NP], F32, tag=tag)
                    d4 = d[:qn, :].rearrange("q (m y p) -> q m y p", m=NH, y=14)
                    nc.vector.tensor_tensor(
                        d4, coord[:qn, :].rearrange("q (m p) -> q m p", m=NH)
                        .unsqueeze(2).broadcast_to((qn, NH, 14, NP)),
                        iota_y, OP.subtract)
                    # tent = relu(1 - |d|), both steps on ACT
                    nc.scalar.activation(d[:qn, :], d[:qn, :], AF.Abs)
                    nc.scalar.activation(d[:qn, :], d[:qn, :], AF.Relu,
                                         bias=1.0, scale=-1.0)
                    return d

                ty = neg_tent(py, "ty")
                tx = neg_tent(px, "tx")
                # fold attention weights into ty
                nc.vector.tensor_tensor(
                    ty[:qn, :].rearrange("q (m y p) -> q m y p", m=NH, y=14),
                    ty[:qn, :].rearrange("q (m y p) -> q m y p", m=NH, y=14),
                    awn[:qn, :].rearrange("q (m p) -> q m p", m=NH)
                    .unsqueeze(2).broadcast_to((qn, NH, 14, NP)), OP.mult)

                # Ws token-major: prod over (y,x) then sum over p
                wst = gpool.tile([128, NH * 196 * NP], F32, tag="wst")
                prod_eng = nc.gpsimd if qc == 0 else nc.vector
                prod_eng.tensor_tensor(
                    wst[:qn, :].rearrange("q (m y x p) -> q m y x p", m=NH, y=14, x=14),
                    ty[:qn, :].rearrange("q (m y p) -> q m y p", m=NH, y=14)
                    .unsqueeze(3).broadcast_to((qn, NH, 14, 14, NP)),
                    tx[:qn, :].rearrange("q (m x p) -> q m x p", m=NH, x=14)
                    .unsqueeze(2).broadcast_to((qn, NH, 14, 14, NP)), OP.mult)
                wsum = rpool.tile([128, NH * 196], F32, tag="wsum")
                nc.vector.tensor_reduce(wsum[:qn, :],
                                        wst[:qn, :].rearrange("q (g p) -> q g p", p=NP),
                                        mybir.AxisListType.X, OP.add)
                WsT.append(wsum)

            # ---- per head: transpose Ws, gather matmuls, token-sum ----
            for h in range(NH):
                # Ws[hw, q] tiles per hw-chunk
                wsh = [hpool.tile([128, LQ], F32, name=f"wsh{f}_{h}_{i}", tag="wsh")
                       for i in range(2)]
                for hc, hn in enumerate(QCH):
                    for qc, qn in enumerate(QCH):
                        tp = pp_t.tile([128, 128], F32, tag="tr")
                        nc.tensor.transpose(
                            tp[:hn, :qn],
                            WsT[qc][:qn, h * 196 + hc * 128: h * 196 + hc * 128 + hn],
                            iden[:qn, :qn])
                        dst = wsh[hc][:hn, qc * 128:qc * 128 + qn]
                        if (hc + qc) % 2 == 0:
                            nc.vector.tensor_copy(dst, tp[:hn, :qn])
                        else:
                            nc.scalar.copy(dst, tp[:hn, :qn])
                # attnT pieces: dh split 128+64 to keep PSUM partitions aligned
                for mc, (m0, mn) in enumerate(((0, 128), (128, 64))):
                    ps = pp_a.tile([128, LQ], F32, tag="mm")
                    for hc, hn in enumerate(QCH):
                        nc.tensor.matmul(
                            ps[:mn, :],
                            V[hc][:hn, h * DH + m0: h * DH + m0 + mn],
                            wsh[hc][:hn, :], start=(hc == 0), stop=(hc == 1))
                    # token-sum into asum rows g0..g0+mn  (g = h*192 + m0 + j)
                    g0 = h * DH + m0
                    r = 0
                    while r < mn:
                        t_i, row = divmod(g0 + r, 128)
                        take = min(mn - r, 128 - row)
                        nc.vector.tensor_reduce(
                            asum[t_i][row:row + take, f:f + 1],
                            ps[r:r + take, :], mybir.AxisListType.X, OP.add)
                        r += take

        # ---- tail: pooled = asum @ woutT (+bo), final = pooled @ wprojT (+bp) ----
        pooled = []
        for m in range(KC):
            ps = pp_a.tile([128, NFRAME], F32, tag="mm")
            for k in range(KC):
                nc.tensor.matmul(ps[:], wout[k][:, m * 128:(m + 1) * 128],
                                 asum[k][:], start=(k == 0), stop=(k == KC - 1))
            pt = spool.tile([128, NFRAME], F32, tag="pool")
            nc.vector.tensor_scalar(pt[:], ps[:], cons[:, C_BOUT + m:C_BOUT + m + 1],
                                    None, OP.add)
            pooled.append(pt)
        for m in range(KC):
            ps = pp_a.tile([128, NFRAME], F32, tag="mm")
            for k in range(KC):
                nc.tensor.matmul(ps[:], wproj[k][:, m * 128:(m + 1) * 128],
                                 pooled[k][:], start=(k == 0), stop=(k == KC - 1))
            ot = spool.tile([128, NFRAME], F32, tag="outt")
            nc.vector.tensor_scalar(ot[:], ps[:], cons[:, C_BPROJ + m:C_BPROJ + m + 1],
                                    None, OP.add)
            nc.sync.dma_start(out8[m], ot[:])


def _host_prep(inputs):
    x = np.asarray(inputs["x"], dtype=np.float32)
    n = x.shape[0] * x.shape[1]
    xi = x.reshape(n, 3, HF, 16, WF, 16)
    xcol = np.ascontiguousarray(xi.transpose(0, 1, 3, 5, 2, 4)).reshape(n, D, LQ)

    wembT = np.ascontiguousarray(
        np.asarray(inputs["embed_w"], np.float32).reshape(D, D).T)
    emb_b = np.asarray(inputs["embed_b"], np.float32)
    value_w = np.asarray(inputs["value_w"], np.float32)
    oa_w = np.concatenate([np.asarray(inputs["off_w"], np.float32),
                           np.asarray(inputs["aw_w"], np.float32)], axis=0)
    # feat only feeds these linears: fold the patch-embed weight in on host
    wvalT = np.ascontiguousarray(wembT @ value_w.T)      # [cpq, dv]
    woaT = np.ascontiguousarray(wembT @ oa_w.T)          # [cpq, 48]
    bv_eff = np.asarray(inputs["value_b"], np.float32) + value_w @ emb_b
    boa_eff = np.concatenate(
        [np.asarray(inputs["off_b"], np.float32),
         np.asarray(inputs["aw_b"], np.float32)]) + oa_w @ emb_b
    woutT = np.ascontiguousarray(
        (np.asarray(inputs["outp_w"], np.float32) / LQ).T)
    wprojT = np.ascontiguousarray(np.asarray(inputs["proj_w"], np.float32).T)

    cons = np.zeros((128, NCONST), np.float32)
    cons[:, C_IOTA:C_IOTA + 14] = np.arange(14, dtype=np.float32)[None, :]
    lin = np.linspace(0.0, 1.0, HF, dtype=np.float32)
    cxf = (14.0 * lin[np.arange(LQ) % WF] - 0.5).astype(np.float32)
    cyf = (14.0 * lin[np.arange(LQ) // WF] - 0.5).astype(np.float32)
    cons[:, C_CX0] = cxf[:128]
    cons[:68, C_CX1] = cxf[128:]
    cons[:, C_CY0] = cyf[:128]
    cons[:68, C_CY1] = cyf[128:]
    cons[:, C_BOUT:C_BOUT + 6] = np.asarray(inputs["outp_b"], np.float32).reshape(6, 128).T
    cons[:, C_BPROJ:C_BPROJ + 6] = np.asarray(inputs["proj_b"], np.float32).reshape(6, 128).T
    cons[0, C_VB:C_VB + D] = bv_eff
    cons[0, C_OAB:C_OAB + 48] = boa_eff

    shared = dict(wvalT=wvalT, woaT=woaT, woutT=woutT,
                  wprojT=wprojT, consts=cons)
    in_maps = [dict(shared, xcol=np.ascontiguousarray(
        xcol[c * NFRAME:(c + 1) * NFRAME])) for c in range(NCORES)]
    return in_maps


_NC_CACHE = None


def _get_nc():
    global _NC_CACHE
    if _NC_CACHE is None:
        _NC_CACHE = _build_program()
    return _NC_CACHE


def kernel(**inputs) -> np.ndarray:
    from concourse.bass_utils import run_bass_kernel_spmd
    nc = _get_nc()
    in_maps = _host_prep(inputs)
    res = run_bass_kernel_spmd(nc, in_maps, list(range(NCORES))).results
    outs = []
    for c in range(NCORES):
        o = res[c]["out8"]              # (6, 128, 8)
        outs.append(o.transpose(2, 0, 1).reshape(NFRAME, D))
    full = np.concatenate(outs, axis=0)  # (64, 768)
    return full.reshape(4, 16, D).astype(np.float32)


# revision 25
# speedup vs baseline: 24.0424x; 1.1254x over previous
"""Trainium2 Bass kernel for DeformableSpatialEncoder.

Math (per frame, Lq=196 tokens on a 14x14 grid, D=768, 4 heads x 192):
  feat   = patch_embed(x)                     -> matmul after host im2col
  value  = feat @ Wv.T                        (token-major on device)
  off,aw = feat @ [Wo;Wa].T (+b), softmax(aw over 4 points)
  attn[q,(m,dh)] = sum_p aw * bilinear_sample(value_m, loc(q,m,p))
  out    = mean_q(attn) @ Wout.T + bo         (mean commutes with linear)
  final  = out @ Wproj.T + bp

Bilinear sampling is exact via "tent" weights: the weight of grid row y for
sample row-coord py is relu(1-|py-y|) (zero outside the grid == zero padding).
So per head  attnT = V.T @ Ws  with  Ws[(y,x), q] = sum_p aw * ty(y) * tx(x).
Ws is built token-major on DVE (per-partition scalars are per-token), then
transposed 128x128-blockwise on the PE.

Sharding: data-parallel over the 64 frames, 8 per core, no collectives.
"""

import os
import sys

for _p in (
    "/root/.axon_site",
    "/root/.axon_site/_ro/trn_rl_repo",
    "/root/.axon_site/_ro/pypackages",
    "/opt/trn_rl_repo",
):
    if os.path.isdir(_p) and _p not in sys.path:
        sys.path.append(_p)

import numpy as np

import concourse.bass as bass
import concourse.mybir as mybir
import concourse.tile as tile
from concourse import bacc
from concourse.masks import make_identity

F32 = mybir.dt.float32
AF = mybir.ActivationFunctionType
OP = mybir.AluOpType

HF = WF = 14
LQ = 196            # tokens per frame
D = 768
NH = 4              # heads
NP = 4              # points
DH = 192            # head dim
NFRAME = 8          # frames per core
NCORES = 8
KC = 6              # 768 / 128 contraction chunks
QCH = (126, 70)     # token chunks; 126 = 9*14 rows so hw-chunks align to grid rows

# consts layout (one [128, NCONST] f32 tensor)
C_IOTA = 0          # cols 0..13: 0..13 in every partition
C_CX0, C_CX1 = 14, 15   # 14*gx - 0.5 per token, chunk0 rows 0..127 / chunk1 rows 0..67
C_CY0, C_CY1 = 16, 17
C_BEMB = 18         # 6 cols: embed_b  per dout-chunk
C_BOUT = 24         # 6 cols: outp_b
C_BPROJ = 30        # 6 cols: proj_b
C_VB = 36           # row 0, cols 36..803: value_b
C_OAB = 804         # row 0, cols 804..851: [off_b; aw_b]
C_IOTA32 = 852      # cols 852..883: 0..13 then 1e9 (tent pad -> exactly 0)
NCONST = 884


def _build_program(reps=1, cut=""):
    nc = bacc.Bacc("TRN2", target_bir_lowering=False, debug=False,
                   enable_asserts=False, num_devices=1)

    xcol = nc.dram_tensor("xcol", (NFRAME, D, LQ), F32, kind="ExternalInput").ap()
    wvoT = nc.dram_tensor("wvoT", (D, D + 48), F32, kind="ExternalInput").ap()
    wtailT = nc.dram_tensor("wtailT", (D, D), F32, kind="ExternalInput").ap()
    consts = nc.dram_tensor("consts", (128, NCONST), F32, kind="ExternalInput").ap()
    out8 = nc.dram_tensor("out8", (KC, 128, NFRAME), F32, kind="ExternalOutput").ap()

    with tile.TileContext(nc) as tc:
        _emit(tc, xcol, wvoT, wtailT, consts, out8, reps, cut)
    nc.compile()
    return nc


def _emit(tc, xcol, wvoT, wtailT, consts, out8, reps=1, cut=""):
    nc = tc.nc
    from contextlib import ExitStack
    ctx = ExitStack()
    with ctx:
        cpool = ctx.enter_context(tc.tile_pool(name="consts", bufs=1))
        wpool = ctx.enter_context(tc.tile_pool(name="weights", bufs=1))
        xpool = ctx.enter_context(tc.tile_pool(name="xin", bufs=12))
        vpool = ctx.enter_context(tc.tile_pool(name="val", bufs=4))
        spool = ctx.enter_context(tc.tile_pool(name="small", bufs=8))
        rpool = ctx.enter_context(tc.tile_pool(name="rall", bufs=4))
        apool = ctx.enter_context(tc.tile_pool(name="accum", bufs=1))
        pp_a = ctx.enter_context(tc.tile_pool(name="psA", bufs=3, space="PSUM"))
        pp_r = ctx.enter_context(tc.tile_pool(name="psR", bufs=2, space="PSUM"))
        pp_s = ctx.enter_context(tc.tile_pool(name="psS", bufs=2, space="PSUM"))

        # ---- constants & weights (loaded once) ----
        cons = cpool.tile([128, NCONST], F32)
        nc.sync.dma_start(cons[:], consts)
        iden = cpool.tile([128, 128], F32)
        make_identity(nc, iden[:])
        vb_bc = cpool.tile([128, D], F32)
        nc.gpsimd.partition_broadcast(vb_bc[:], cons[0:1, C_VB:C_VB + D])
        oab_bc = cpool.tile([128, 48], F32)
        nc.gpsimd.partition_broadcast(oab_bc[:], cons[0:1, C_OAB:C_OAB + 48])

        def load_w(nm, ap_dram, width):
            ts_ = []
            for k in range(KC):
                t = wpool.tile([128, width], F32, name=f"{nm}{k}", tag=f"{nm}{k}")
                eng = nc.sync if k % 2 == 0 else nc.scalar
                eng.dma_start(t[:], ap_dram[k * 128:(k + 1) * 128, :])
                ts_.append(t)
            return ts_

        wvo = load_w("wv", wvoT, D + 48)
        wtail = load_w("wt", wtailT, D)

        # attn token-sum rows, one row per frame
        frames8 = apool.tile([NFRAME, D], F32, name="frames8", tag="frames8")
        if cut:
            nc.vector.memset(frames8[:], 0.0)

        for f in [fr for _ in range(reps) for fr in range(NFRAME)]:
            # ---- load im2col'd frame ----
            xin = []
            for k in range(KC):
                t = xpool.tile([128, LQ], F32)
                eng = nc.sync if k % 2 == 0 else nc.scalar
                eng.dma_start(t[:], xcol[f, k * 128:(k + 1) * 128, :])
                xin.append(t)

            # ---- fused value+off/aw matmuls, token-major [q, 768+48] ----
            V = []
            OA = []
            for qc, qn in enumerate(QCH):
                vt = vpool.tile([128, D], F32, tag="v")
                q0 = qc * QCH[0]
                qs = slice(q0, q0 + qn)
                ps1 = pp_a.tile([128, 512], F32, tag="mm")
                ps2 = pp_a.tile([128, 304], F32, tag="mm")
                for k in range(KC):
                    nc.tensor.matmul(ps1[:qn, :], xin[k][:, qs], wvo[k][:, 0:512],
                                     start=(k == 0), stop=(k == KC - 1))
                for k in range(KC):
                    nc.tensor.matmul(ps2[:qn, :], xin[k][:, qs], wvo[k][:, 512:816],
                                     start=(k == 0), stop=(k == KC - 1))
                nc.vector.tensor_tensor(vt[:qn, 0:512], ps1[:qn, :],
                                        vb_bc[:qn, 0:512], OP.add)
                nc.vector.tensor_tensor(vt[:qn, 512:768], ps2[:qn, 0:256],
                                        vb_bc[:qn, 512:768], OP.add)
                oa = spool.tile([128, 48], F32, tag="oa")
                nc.vector.tensor_tensor(oa[:qn, :], ps2[:qn, 256:304],
                                        oab_bc[:qn, :], OP.add)
                V.append(vt)
                OA.append(oa)

            if cut == "after_val":
                rowc = spool.tile([1, 2], F32, tag="rowc")
                for qc, qn in enumerate(QCH):
                    nc.vector.tensor_copy(rowc[0:1, qc:qc + 1], V[qc][0:1, 0:1])
                nc.sync.dma_start(frames8[f:f + 1, 0:2], rowc[0:1, :])
                continue

            # ---- off/aw -> softmax -> tents (token-major) ----
            TY = []
            TX = []
            for qc, qn in enumerate(QCH):
                oa = OA[qc]

                # softmax over points: aw cols 32..48 viewed [q, m, p]
                aw3 = oa[:qn, 32:48].rearrange("q (m p) -> q m p", m=NH)
                mx = spool.tile([128, NH], F32, tag="mx")
                nc.vector.tensor_reduce(mx[:qn, :], aw3, mybir.AxisListType.X, OP.max)
                ex = spool.tile([128, 16], F32, tag="ex")
                nc.vector.tensor_tensor(
                    ex[:qn, :].rearrange("q (m p) -> q m p", m=NH), aw3,
                    mx[:qn, :].unsqueeze(2).broadcast_to((qn, NH, NP)), OP.subtract)
                nc.scalar.activation(ex[:qn, :], ex[:qn, :], AF.Exp)
                sm = spool.tile([128, NH], F32, tag="sm")
                nc.vector.tensor_reduce(sm[:qn, :],
                                        ex[:qn, :].rearrange("q (m p) -> q m p", m=NH),
                                        mybir.AxisListType.X, OP.add)
                nc.vector.reciprocal(sm[:qn, :], sm[:qn, :])
                awn = spool.tile([128, 16], F32, tag="awn")
                nc.vector.tensor_tensor(
                    awn[:qn, :].rearrange("q (m p) -> q m p", m=NH),
                    ex[:qn, :].rearrange("q (m p) -> q m p", m=NH),
                    sm[:qn, :].unsqueeze(2).broadcast_to((qn, NH, NP)), OP.mult)

                # sample coords: px = offx + (14*gx - 0.5), py likewise
                cx = cons[:qn, C_CX0 + qc:C_CX0 + qc + 1]
                cy = cons[:qn, C_CY0 + qc:C_CY0 + qc + 1]
                px = spool.tile([128, 16], F32, tag="px")
                py = spool.tile([128, 16], F32, tag="py")
                offv = oa[:qn, 0:32].rearrange("q (m p c) -> q m p c", m=NH, p=NP)
                nc.vector.tensor_scalar(px[:qn, :].rearrange("q (m p) -> q m p", m=NH),
                                        offv[:, :, :, 0], cx, None, OP.add)
                nc.vector.tensor_scalar(py[:qn, :].rearrange("q (m p) -> q m p", m=NH),
                                        offv[:, :, :, 1], cy, None, OP.add)

                # tents in padded layout [q, (m, p, 32)]: head h's operand is the
                # contiguous 128-col window; pad iota is 1e9 so pad tents are 0
                iota_y = cons[:qn, C_IOTA32:C_IOTA32 + 32] \
                    .unsqueeze(1).unsqueeze(2).broadcast_to((qn, NH, NP, 32))

                def mk_tent(coord, tag):
                    d = spool.tile([128, 512], F32, tag=tag)
                    d4 = d[:qn, :].rearrange("q (m p y) -> q m p y", m=NH, p=NP)
                    nc.vector.tensor_tensor(
                        d4, coord[:qn, :].rearrange("q (m p) -> q m p", m=NH)
                        .unsqueeze(3).broadcast_to((qn, NH, NP, 32)),
                        iota_y, OP.subtract)
                    # tent = relu(1 - |d|), both steps on ACT
                    nc.scalar.activation(d[:qn, :], d[:qn, :], AF.Abs)
                    nc.scalar.activation(d[:qn, :], d[:qn, :], AF.Relu,
                                         bias=1.0, scale=-1.0)
                    return d

                ty = mk_tent(py, "ty")
                tx = mk_tent(px, "tx")
                # fold attention weights into ty (tents are positive now)
                nc.vector.tensor_tensor(
                    ty[:qn, :].rearrange("q (m p y) -> q m p y", m=NH, p=NP),
                    ty[:qn, :].rearrange("q (m p y) -> q m p y", m=NH, p=NP),
                    awn[:qn, :].rearrange("q (m p) -> q m p", m=NH)
                    .unsqueeze(3).broadcast_to((qn, NH, NP, 32)), OP.mult)
                TY.append(ty)
                TX.append(tx)

            # ---- r_m[y, x] = sum_{q,p} (aw*ty)[q,m,y,p] * tx[q,m,x,p] ----
            # (only the token-MEAN of the attention output is ever needed,
            #  so the whole deformable gather collapses to V.T @ r)
            rall = [rpool.tile([128, NH], F32, name=f"rall{f}_{hc}", tag=f"rl{hc}")
                    for hc in range(2)]
            rsb4 = spool.tile([14, NH * 14], F32, tag="rsb4")
            for h in range(NH):
                pr = pp_r.tile([32, 32], F32, tag="r")
                for qc, qn in enumerate(QCH):
                    for p in range(NP):
                        ws = slice(h * 128 + 32 * p, h * 128 + 32 * (p + 1))
                        nc.tensor.matmul(pr[:, :], TY[qc][:qn, ws], TX[qc][:qn, ws],
                                         start=(qc == 0 and p == 0),
                                         stop=(qc == 1 and p == NP - 1))
                nc.vector.tensor_copy(rsb4[:, h:h + 4 * 13 + 1:NH], pr[0:14, 0:14])
            # scatter r rows into hw-partition columns (126 = 9 grid rows)
            nc.sync.dma_start(rall[0][0:126, :], rsb4[0:9, :])
            nc.scalar.dma_start(rall[1][0:70, :], rsb4[9:14, :])

            # ---- attn token-sum: asum[m-block] = V.T @ r_m  (M=1 per head) ----
            row = spool.tile([1, D], F32, tag="row")
            for m in range(NH):
                pas = pp_s.tile([1, DH], F32, tag="as")
                for hc, hn in enumerate(QCH):
                    nc.tensor.matmul(pas[:, :], rall[hc][:hn, m:m + 1],
                                     V[hc][:hn, m * DH:(m + 1) * DH],
                                     start=(hc == 0), stop=(hc == 1))
                nc.vector.tensor_copy(row[0:1, m * DH:(m + 1) * DH], pas[:, :])
            nc.sync.dma_start(frames8[f:f + 1, :], row[0:1, :])

        # ---- tail: asum = frames8.T ; final = asum @ wtailT (+b_tail) ----
        asum = []
        for t in range(KC):
            tp = pp_s.tile([128, NFRAME], F32, tag="as")
            nc.tensor.transpose(tp[:, :], frames8[:, t * 128:(t + 1) * 128],
                                iden[0:NFRAME, 0:NFRAME])
            at = spool.tile([128, NFRAME], F32, tag=f"at{t}", name=f"at{t}")
            nc.vector.tensor_copy(at[:], tp[:, :])
            asum.append(at)
        for m in range(KC):
            ps = pp_a.tile([128, NFRAME], F32, tag="mm")
            for k in range(KC):
                nc.tensor.matmul(ps[:], wtail[k][:, m * 128:(m + 1) * 128],
                                 asum[k][:], start=(k == 0), stop=(k == KC - 1))
            ot = spool.tile([128, NFRAME], F32, tag="outt")
            nc.vector.tensor_scalar(ot[:], ps[:], cons[:, C_BPROJ + m:C_BPROJ + m + 1],
                                    None, OP.add)
            nc.sync.dma_start(out8[m], ot[:])


def _host_prep(inputs):
    x = np.asarray(inputs["x"], dtype=np.float32)
    n = x.shape[0] * x.shape[1]
    xi = x.reshape(n, 3, HF, 16, WF, 16)
    xcol = np.ascontiguousarray(xi.transpose(0, 1, 3, 5, 2, 4)).reshape(n, D, LQ)

    wembT = np.ascontiguousarray(
        np.asarray(inputs["embed_w"], np.float32).reshape(D, D).T)
    emb_b = np.asarray(inputs["embed_b"], np.float32)
    value_w = np.asarray(inputs["value_w"], np.float32)
    oa_w = np.concatenate([np.asarray(inputs["off_w"], np.float32),
                           np.asarray(inputs["aw_w"], np.float32)], axis=0)
    # feat only feeds the value/off/aw linears: fold patch-embed in on host,
    # and fuse value | off | aw into one [cpq, 816] weight
    wvoT = np.ascontiguousarray(
        np.concatenate([wembT @ value_w.T, wembT @ oa_w.T], axis=1))
    bv_eff = np.asarray(inputs["value_b"], np.float32) + value_w @ emb_b
    boa_eff = np.concatenate(
        [np.asarray(inputs["off_b"], np.float32),
         np.asarray(inputs["aw_b"], np.float32)]) + oa_w @ emb_b
    # mean commutes with outp/proj: final = asum @ ((proj @ outp)/LQ).T + b_tail
    outp_w = np.asarray(inputs["outp_w"], np.float32)
    proj_w = np.asarray(inputs["proj_w"], np.float32)
    wtailT = np.ascontiguousarray(((proj_w @ outp_w) / LQ).T)
    b_tail = proj_w @ np.asarray(inputs["outp_b"], np.float32) \
        + np.asarray(inputs["proj_b"], np.float32)

    cons = np.zeros((128, NCONST), np.float32)
    cons[:, C_IOTA:C_IOTA + 14] = np.arange(14, dtype=np.float32)[None, :]
    i32 = np.full(32, 1e9, np.float32)
    i32[:14] = np.arange(14, dtype=np.float32)
    cons[:, C_IOTA32:C_IOTA32 + 32] = i32[None, :]
    lin = np.linspace(0.0, 1.0, HF, dtype=np.float32)
    cxf = (14.0 * lin[np.arange(LQ) % WF] - 0.5).astype(np.float32)
    cyf = (14.0 * lin[np.arange(LQ) // WF] - 0.5).astype(np.float32)
    cons[:126, C_CX0] = cxf[:126]
    cons[:70, C_CX1] = cxf[126:]
    cons[:126, C_CY0] = cyf[:126]
    cons[:70, C_CY1] = cyf[126:]
    cons[:, C_BPROJ:C_BPROJ + 6] = b_tail.reshape(6, 128).T
    cons[0, C_VB:C_VB + D] = bv_eff
    cons[0, C_OAB:C_OAB + 48] = boa_eff

    shared = dict(wvoT=wvoT, wtailT=wtailT, consts=cons)
    in_maps = [dict(shared, xcol=np.ascontiguousarray(
        xcol[c * NFRAME:(c + 1) * NFRAME])) for c in range(NCORES)]
    return in_maps


_NC_CACHE = None


def _get_nc():
    global _NC_CACHE
    if _NC_CACHE is None:
        _NC_CACHE = _build_program()
    return _NC_CACHE


def kernel(**inputs) -> np.ndarray:
    from concourse.bass_utils import run_bass_kernel_spmd
    nc = _get_nc()
    in_maps = _host_prep(inputs)
    res = run_bass_kernel_spmd(nc, in_maps, list(range(NCORES))).results
    outs = []
    for c in range(NCORES):
        o = res[c]["out8"]              # (6, 128, 8)
        outs.append(o.transpose(2, 0, 1).reshape(NFRAME, D))
    full = np.concatenate(outs, axis=0)  # (64, 768)
    return full.reshape(4, 16, D).astype(np.float32)
